# revision 62
# baseline (speedup 1.0000x reference)
"""Masked attention (B=16, QT=KT=2048, D=1024, fp32) on 8 Trainium2 NeuronCores.

Strategy:
 - Work unit = (128 q rows) x (up to 512 k cols) partial attention with online
   (flash-style) softmax accumulation; k-outer / q-inner inside a "fragment".
 - K-length specialization at 128-col granularity: a fragment processing a
   batch with KB = ceil(K_len/128) 128-blocks executes full 512-wide chunks
   plus one final variable-width chunk (128/256/384), cutting both matmul rows
   and K/V DMA to the actual length.
 - A fragment = (NQ q-tiles) x (KB k-blocks) of one batch; every core runs an
   identical static sequence of fragment shapes (SPMD), host packs which
   (batch, q-range) goes where, padding with dummy slots. The slot-shape plan
   comes from a DFS that minimizes max(PE time, DMA time): PE pays KB*NQ
   k128-units per slot (incl. padding), DMA pays KB chunk-loads.
 - Per-chunk DRAM record [128 part, 17, 512] fp16 = 8 K d-chunks + mask row
   + 8 V (kb, dh) columns, packed host-side partition-major so every DMA line
   is 2-17KB contiguous (few, large packets), split into a k+mask DMA (gates
   QK) and a V DMA (gates PV only).
 - Engines: PE does QK / P-transpose / PV matmuls only; DVE adds the mask
   (draining S PSUM), takes the running max, and rescales the O accumulator;
   Scalar does exp (+row sums) and finalization; Pool only dispatches Q DMAs
   (its compute datapath is a slow software DSP - do not offload math there;
   it also cannot touch PSUM).
 - Numerics: Q/K/V/P in fp16 (S accumulated in fp32 PSUM, softmax in fp32),
   output fp16 (quantization ~5e-4 vs |O|max, halves output DMA). End-to-end
   absmax relative error vs the fp32 reference ~1.1e-2 (gate is 2e-2).
   Invalid q rows fixed on host (uniform average of V).
"""

import os
import numpy as np
from contextlib import ExitStack

import concourse.bass as bass
import concourse.tile as tile
from concourse import bacc, mybir
from concourse.bass_utils import run_bass_kernel_spmd

F32 = mybir.dt.float32
FP16 = mybir.dt.float16
AF = mybir.ActivationFunctionType
ALU = mybir.AluOpType

B, QT, KT, D = 16, 2048, 2048, 1024
QTILE = 128
KCH = 512               # max chunk width
NCORES = 8
DCH = D // 128          # 8 contraction chunks of 128
KBLK = KCH // 128       # max k sub-blocks per chunk
MASKVAL = -60000.0      # fp16-exact; exp(S+mask-m) underflows to 0 exactly
NQ_MAX = 8
WARMUP_MM = 26

_PROG_CACHE: dict = {}
LAST_EXEC_NS = [None]

# planner time model (ns): per k128-unit PE, per k128-chunk DMA, fixed terms
PE_UNIT_NS = 920.0
DMA_CHUNK_NS = float(os.environ.get("ATTN_DMA_CHUNK_NS", "1750"))
PE_FIXED_NS = 4500.0
DMA_FIXED_NS = 28000.0


def _widths(kb):
    """Chunk widths (in cols) for kb 128-blocks: full 512s + one partial."""
    ws = [KCH] * (kb // KBLK)
    if kb % KBLK:
        ws.append(128 * (kb % KBLK))
    return ws


def _group_widths(gi, kb):
    """Chunk widths for group gi (uniform; a leading-chunk split was tried
    to cut the first QK's DMA wait but regressed ~3.6us: the extra chunk's
    per-tile softmax instances and fragmented early DMA cost more)."""
    return _widths(kb)


# --------------------------------------------------------------------------
# planning: choose fragment shape classes + assign (batch, q-run) fragments
# --------------------------------------------------------------------------

def _fill_slot(kb, nq, rem, nk, min_frac=0.0):
    """Greedily fill the NCORES positions of a (kb, nq) slot from remaining
    tile demand (batches with nk <= kb, largest covered-work first). A first
    pass only admits batches with nk >= min_frac*kb (avoids stranding deep
    batches while shallow ones steal deep slots); leftover positions are
    filled unrestricted. Returns (placed [(b, ln)...], new_rem)."""
    rem = dict(rem)
    placed = []
    for lo in (min_frac * kb, 0.0) if min_frac else (0.0,):
        while len(placed) < NCORES:
            cands = [b for b in rem if lo <= nk[b] <= kb]
            if not cands:
                break
            b = max(cands, key=lambda b: (min(rem[b], nq) * nk[b], nk[b]))
            ln = min(rem[b], nq)
            placed.append((b, ln))
            rem[b] -= ln
            if rem[b] == 0:
                del rem[b]
    return placed, rem


def _plan(nqt, nk):
    """DFS over slot shapes: each slot (KB, NQ) is executed by all 8 cores
    (8 positions, one single-batch run of <=NQ tiles per position). The
    objective is span = max(PE time, DMA time): PE pays KB*NQ k128-units per
    slot (incl. padding), DMA pays KB chunk-loads per slot.
    Returns (groups [(KB, NQ, 1)...], assign {(core, gi, 0): (b, q0, ln)})."""
    rem0 = {b: nqt[b] for b in range(len(nqt)) if nqt[b] > 0}
    if not rem0:
        return [], {}
    best = [None, float("inf")]
    nodes = [0]

    def objective(pe, dma):
        pe_ns = PE_UNIT_NS * pe + PE_FIXED_NS
        dma_ns = DMA_CHUNK_NS * dma + DMA_FIXED_NS
        return max(pe_ns, dma_ns) + 0.1 * min(pe_ns, dma_ns)

    def dfs(rem, slots, pe, dma):
        nodes[0] += 1
        if nodes[0] > 300000:
            return
        if not rem:
            obj = objective(pe, dma)
            if obj < best[1]:
                best[0], best[1] = list(slots), obj
            return
        lb_pe = sum(rem[b] * nk[b] for b in rem) / NCORES
        lb_dma = sum(nk[b] * -(-rem[b] // NQ_MAX) for b in rem) / NCORES
        if objective(pe + lb_pe, dma + lb_dma) >= best[1]:
            return
        bmax = max(rem, key=lambda b: (nk[b], rem[b]))
        kb = nk[bmax]
        opts, seen = [], set()
        for nq in range(1, NQ_MAX + 1):
            for mf in (0.0, 0.75, 1.0):
                placed, rem2 = _fill_slot(kb, nq, rem, nk, mf)
                if not placed:
                    continue
                key = tuple(sorted(placed))
                if key in seen:
                    continue
                seen.add(key)
                covered = sum(ln * nk[b] for (b, ln) in placed)
                score = covered / (kb * nq + 1.9 * kb)
                opts.append((score, nq, placed, rem2))
        opts.sort(reverse=True, key=lambda o: o[0])
        for (_, nq, placed, rem2) in opts:
            dfs(rem2, slots + [(kb, nq, placed)], pe + kb * nq, dma + kb)

    dfs(rem0, [], 0.0, 0.0)
    assert best[0] is not None, "no feasible plan"

    # the most compute-dense slot (highest NQ) first to prime the pipeline
    # (most PE work per DMA byte while queues ramp), then large-KB first;
    # small fragments at the tail ride on already-prefetched DMA
    slots = sorted(best[0], key=lambda s: (-s[0], -s[1]))
    lead = max(range(len(slots)), key=lambda i: (slots[i][1], slots[i][0]))
    slots = [slots[lead]] + slots[:lead] + slots[lead + 1:]
    groups = [(kb, nq, 1) for (kb, nq, _) in slots]

    # convert per-slot (b, ln) lists into runs with q offsets, then assign
    # positions to cores balancing cumulative real work
    offs = {b: 0 for b in rem0}
    load = [0.0] * NCORES
    assign = {}
    for gi, (kb, nq, placed) in enumerate(slots):
        runs = []
        for (b, ln) in placed:
            runs.append((b, offs[b], ln))
            offs[b] += ln
        runs.sort(key=lambda r: -(r[2] * nk[r[0]]))
        order = sorted(range(NCORES), key=lambda c: load[c])
        for i, (b, q0, ln) in enumerate(runs):
            c = order[i]
            assign[(c, gi, 0)] = (b, q0, ln)
            load[c] += ln * nk[b]
    return groups, assign


# --------------------------------------------------------------------------
# device program (cached by fragment-shape signature)
# --------------------------------------------------------------------------

def _build_program(groups):
    TQ = sum(NQ * F for (_, NQ, F) in groups)
    CH = sum(len(_group_widths(gi, KB)) * F
             for gi, (KB, _, F) in enumerate(groups))

    nc = bacc.Bacc("TRN2", target_bir_lowering=False, debug=False)
    # partition-major contiguous layouts: per partition line 2-17KB.
    # kv record free layout: [0:8]=K d-chunks, [8]=mask row, [9:17]=V pairs
    qh_e = nc.dram_tensor("qh", [TQ, 128, DCH, QTILE], FP16, kind="ExternalInput")
    kv_e = nc.dram_tensor("kv", [CH, 128, 2 * DCH + 1, KCH], FP16, kind="ExternalInput")
    id_e = nc.dram_tensor("ident", [128, 128], FP16, kind="ExternalInput")
    o_e = nc.dram_tensor("o", [TQ, 128, D], FP16, kind="ExternalOutput")

    with tile.TileContext(nc) as tc:
        with ExitStack() as ctx:
            const = ctx.enter_context(tc.tile_pool(name="const", bufs=1))
            deep = 6
            qpool = ctx.enter_context(tc.tile_pool(name="qpool", bufs=3))
            kvpool = ctx.enter_context(tc.tile_pool(name="kvpool", bufs=deep))
            state = ctx.enter_context(tc.tile_pool(name="state", bufs=2))
            work = ctx.enter_context(tc.tile_pool(name="work", bufs=3))
            small = ctx.enter_context(tc.tile_pool(name="small", bufs=6))
            opool = ctx.enter_context(tc.tile_pool(name="opool", bufs=2))
            ps_s = ctx.enter_context(tc.tile_pool(name="ps_s", bufs=2, space="PSUM"))
            ps_t = ctx.enter_context(tc.tile_pool(name="ps_t", bufs=2, space="PSUM"))
            ps_o = ctx.enter_context(tc.tile_pool(name="ps_o", bufs=2, space="PSUM"))

            ident = const.tile([128, 128], FP16)
            nc.sync.dma_start(ident[:], id_e[:])
            # HAM warm-up: dummy matmuls during the initial DMA ramp so the
            # PE clock is at full p-state when real work starts
            for w in range(WARMUP_MM):
                wp = ps_t.tile([128, 128], F32, tag="ptp")
                nc.tensor.matmul(wp[:], ident[:], ident[:], start=True,
                                 stop=True)

            qslot = 0
            chslot = 0
            for gi, (KB, NQ, F) in enumerate(groups):
                ws = _group_widths(gi, KB)
                NKC = len(ws)
                for f in range(F):
                    # fragment state (not needed for single-chunk fragments)
                    if NKC > 1:
                        mbar = state.tile([128, NQ], F32, tag="mbar")
                        dst = state.tile([128, NQ], F32, tag="dst")
                        oacc = state.tile([128, NQ * D], F32, tag="oacc")

                    # load this fragment's q tiles
                    qh = qpool.tile([128, NQ, DCH, QTILE], FP16, tag="qh")
                    for t in range(NQ):
                        nc.gpsimd.dma_start(qh[:, t], qh_e[qslot + t])

                    for j, w in enumerate(ws):
                        nb = w // 128
                        # k+mask and v as separate DMAs: QK/mask-add only
                        # wait on the first, PV on the second
                        kv = kvpool.tile([128, 2 * DCH + 1, KCH], FP16, tag="kv")
                        if w == KCH:
                            nc.sync.dma_start(kv[:, :DCH + 1],
                                              kv_e[chslot + j][:, :DCH + 1])
                            nc.sync.dma_start(kv[:, DCH + 1:],
                                              kv_e[chslot + j][:, DCH + 1:])
                        else:
                            nc.sync.dma_start(kv[:, :DCH + 1, :w],
                                              kv_e[chslot + j][:, :DCH + 1, :w])
                            nc.sync.dma_start(
                                kv[:, DCH + 1:DCH + 1 + 2 * nb],
                                kv_e[chslot + j][:, DCH + 1:DCH + 1 + 2 * nb])

                        for t in range(NQ):
                            # S accumulation in fp32 PSUM
                            sp = ps_s.tile([128, KCH], F32, tag="sp")
                            for c in range(DCH):
                                nc.tensor.matmul(
                                    sp[:, :w], qh[:, t, c], kv[:, c, :w],
                                    start=(c == 0), stop=(c == DCH - 1))

                            # additive length mask (DVE reads PSUM,
                            # releases the S PSUM bank early)
                            s_sb = work.tile([128, KCH], F32, tag="s_sb")
                            nc.vector.tensor_add(s_sb[:, :w], sp[:, :w],
                                                 kv[:, DCH, :w])
                            mbj = small.tile([128, 1], F32, tag="mbj")
                            nc.vector.tensor_reduce(
                                mbj[:], s_sb[:, :w], axis=mybir.AxisListType.X,
                                op=ALU.max, negate=True)

                            st = slice(t, t + 1)
                            if j == 0:
                                if NKC > 1:
                                    nc.vector.tensor_copy(mbar[:, st], mbj[:])
                                mnew = mbj
                            else:
                                mnew = small.tile([128, 1], F32, tag="mnew")
                                nc.vector.tensor_tensor(
                                    mnew[:], mbj[:], mbar[:, st], ALU.min)
                                alpha = small.tile([128, 1], F32, tag="alpha")
                                # alpha = exp(m_old - m_new) = exp(mnew_bar - mold_bar)
                                nc.scalar.activation(
                                    alpha[:], mbar[:, st], AF.Exp,
                                    bias=mnew[:], scale=-1.0)
                                if j < NKC - 1:
                                    nc.vector.tensor_copy(mbar[:, st], mnew[:])

                            # P = exp(S - m), row sums
                            p_sb = work.tile([128, KCH], FP16, tag="p_sb")
                            sj = small.tile([128, 1], F32, tag="sj")
                            nc.scalar.activation(
                                p_sb[:, :w], s_sb[:, :w], AF.Exp, bias=mnew[:],
                                scale=1.0, accum_out=sj[:])

                            if NKC > 1:
                                if j == 0:
                                    nc.vector.tensor_copy(dst[:, st], sj[:])
                                else:
                                    nc.vector.scalar_tensor_tensor(
                                        out=dst[:, st], in0=dst[:, st],
                                        scalar=alpha[:], in1=sj[:],
                                        op0=ALU.mult, op1=ALU.add)

                            # transpose P blocks into one PSUM tile, drain
                            # with a single DVE copy (fewer small DVE ops on
                            # the PE-critical T->copy->PV edge)
                            ptp = ps_t.tile([128, KBLK, 128], FP16, tag="ptp")
                            for kb in range(nb):
                                nc.tensor.transpose(
                                    ptp[:, kb], p_sb[:, bass.ts(kb, 128)],
                                    ident[:])
                            pt = work.tile([128, KBLK, 128], FP16, tag="pt")
                            nc.vector.tensor_copy(pt[:, :nb], ptp[:, :nb])

                            # O_j = P^T-blocks @ V
                            op = ps_o.tile([128, D], F32, tag="op")
                            for dh in range(2):
                                for kb in range(nb):
                                    nc.tensor.matmul(
                                        op[:, bass.ds(dh * 512, 512)],
                                        pt[:, kb],
                                        kv[:, DCH + 1 + 2 * kb + dh],
                                        start=(kb == 0), stop=(kb == nb - 1))

                            ot = slice(t * D, (t + 1) * D)
                            if NKC == 1:
                                # single-chunk fragment: finalize straight
                                # from PSUM (no accumulator round-trip)
                                rec = small.tile([128, 1], F32, tag="rec")
                                nc.vector.reciprocal(rec[:], sj[:])
                                ofin = opool.tile([128, D], FP16, tag="ofin")
                                nc.scalar.activation(
                                    ofin[:], op[:], AF.Copy, bias=0.0,
                                    scale=rec[:])
                                nc.sync.dma_start(o_e[qslot + t], ofin[:])
                                continue
                            if j == 0:
                                nc.scalar.copy(oacc[:, ot], op[:])
                            else:
                                nc.vector.scalar_tensor_tensor(
                                    out=oacc[:, ot], in0=oacc[:, ot],
                                    scalar=alpha[:], in1=op[:],
                                    op0=ALU.mult, op1=ALU.add)
                            if j == NKC - 1:
                                # finalize this q-tile now: overlaps with the
                                # remaining tiles' compute instead of stacking
                                # at the fragment end
                                rec = small.tile([128, 1], F32, tag="rec")
                                nc.vector.reciprocal(rec[:], dst[:, st])
                                ofin = opool.tile([128, D], FP16, tag="ofin")
                                nc.scalar.activation(
                                    ofin[:], oacc[:, ot], AF.Copy, bias=0.0,
                                    scale=rec[:])
                                nc.sync.dma_start(o_e[qslot + t], ofin[:])

                    qslot += NQ
                    chslot += NKC

    nc.compile()
    return nc, TQ, CH


# --------------------------------------------------------------------------
# cached PJRT executor (adapted from concourse.bass2jax.run_bass_via_pjrt)
# --------------------------------------------------------------------------

_EXEC_CACHE: dict = {}


def _get_exec(nc):
    import jax
    from concourse import bass2jax, mybir as _mb
    from jax.experimental.shard_map import shard_map
    from jax.sharding import Mesh, PartitionSpec

    key = id(nc)
    if key in _EXEC_CACHE:
        return _EXEC_CACHE[key]
    bass2jax.install_neuronx_cc_hook()
    assert not nc.dbg_addr or not nc.dbg_callbacks

    partition_name = nc.partition_id_tensor.name if nc.partition_id_tensor else None
    in_names, out_names, out_avals = [], [], []
    for alloc in nc.m.functions[0].allocations:
        if not isinstance(alloc, _mb.MemoryLocationSet):
            continue
        name = alloc.memorylocations[0].name
        if alloc.kind == "ExternalInput":
            if name != partition_name:
                in_names.append(name)
        elif alloc.kind == "ExternalOutput":
            shape = tuple(alloc.tensor_shape)
            dtype = _mb.dt.np(alloc.dtype)
            out_names.append(name)
            out_avals.append(jax.core.ShapedArray(shape, dtype))
    n_params = len(in_names)
    n_outs = len(out_avals)
    all_in_names = list(in_names) + list(out_names)
    if partition_name is not None:
        all_in_names.append(partition_name)
    donate = tuple(range(n_params, n_params + n_outs))

    def _body(*args):
        operands = list(args)
        if partition_name is not None:
            operands.append(bass2jax.partition_id_tensor())
        return tuple(bass2jax._bass_exec_p.bind(
            *operands,
            out_avals=tuple(out_avals),
            in_names=tuple(all_in_names),
            out_names=tuple(out_names),
            lowering_input_output_aliases=(),
            sim_require_finite=True,
            sim_require_nnan=True,
            nc=nc,
        ))

    devices = jax.devices()[:NCORES]
    mesh = Mesh(np.asarray(devices), ("core",))
    in_specs = (PartitionSpec("core"),) * (n_params + n_outs)
    out_specs = (PartitionSpec("core"),) * n_outs
    sharded = jax.jit(
        shard_map(_body, mesh=mesh, in_specs=in_specs, out_specs=out_specs,
                  check_rep=False),
        donate_argnums=donate, keep_unused=True)
    info = dict(sharded=sharded, in_names=in_names, out_names=out_names,
                out_avals=out_avals, mesh=mesh, n_params=n_params)
    _EXEC_CACHE[key] = info
    return info


def _concat_inputs(info, in_maps):
    return [np.concatenate([np.asarray(m[name]) for m in in_maps], axis=0)
            for name in info["in_names"]]


def _zero_outs(info):
    return [np.zeros((NCORES * a.shape[0], *a.shape[1:]), a.dtype)
            for a in info["out_avals"]]


def _execute(nc, in_maps):
    try:
        info = _get_exec(nc)
        concat_in = _concat_inputs(info, in_maps)
        out_arrs = info["sharded"](*concat_in, *_zero_outs(info))
        results = [
            {name: np.asarray(out_arrs[i]).reshape(
                NCORES, *info["out_avals"][i].shape)[c]
             for i, name in enumerate(info["out_names"])}
            for c in range(NCORES)
        ]
        if int(os.environ.get("ATTN_TIME", "0")):
            LAST_EXEC_NS[0] = _time_exec(
                nc, concat_in, int(os.environ.get("ATTN_TIME_ITERS", "3")))
        return results
    except Exception:
        # robust fallback: the canonical entry point (same underlying path,
        # uncached) — also covers non-axon native environments
        res = run_bass_kernel_spmd(nc, in_maps, core_ids=list(range(NCORES)))
        return res.results


def _time_exec(nc, concat_in, iters=3):
    """Wall-clock the sharded execution with device-resident inputs."""
    import time
    import jax
    from jax.sharding import NamedSharding, PartitionSpec
    info = _get_exec(nc)
    sh = NamedSharding(info["mesh"], PartitionSpec("core"))
    dev_in = [jax.device_put(x, sh) for x in concat_in]
    for x in dev_in:
        x.block_until_ready()
    times = []
    for _ in range(iters):
        zeros = [jax.device_put(z, sh) for z in _zero_outs(info)]
        for z in zeros:
            z.block_until_ready()
        t0 = time.perf_counter()
        outs = info["sharded"](*dev_in, *zeros)
        for o in outs:
            o.block_until_ready()
        times.append(time.perf_counter() - t0)
    best = min(times)
    print(f"exec wall times: {[f'{t*1e3:.2f}ms' for t in times]}")
    return int(best * 1e9)


# --------------------------------------------------------------------------
# host entry
# --------------------------------------------------------------------------

def kernel(Q, K, V, Q_lengths, K_lengths):
    Q = np.ascontiguousarray(np.asarray(Q, dtype=np.float32))
    K = np.ascontiguousarray(np.asarray(K, dtype=np.float32))
    V = np.ascontiguousarray(np.asarray(V, dtype=np.float32))
    ql_i = np.asarray(Q_lengths).astype(np.int64)
    kl_i = np.asarray(K_lengths).astype(np.int64)

    nqt = [int(-(-min(max(q, 0), QT) // QTILE)) for q in ql_i]
    nk = [int(-(-min(max(k, 1), KT) // 128)) for k in kl_i]

    groups, assign = _plan(nqt, nk)
    sig = tuple(groups)
    if sig not in _PROG_CACHE:
        _PROG_CACHE[sig] = _build_program(groups)
    nc, TQ, CH = _PROG_CACHE[sig]

    Qh = Q.astype(np.float16)
    Kh = K.astype(np.float16)
    Vr = V.astype(np.float16)

    def pack_core(run_for):
        qh_a = np.zeros((TQ, 128, DCH, QTILE), dtype=np.float16)
        kv_a = np.zeros((CH, 128, 2 * DCH + 1, KCH), dtype=np.float16)
        kv_a[:, :, DCH, :] = MASKVAL
        qslot = chslot = 0
        for gi, (KB, NQ, F) in enumerate(groups):
            ws = _group_widths(gi, KB)
            for f in range(F):
                run = run_for(gi, f)
                if run is not None:
                    b, q0, ln = run
                    for t in range(ln):
                        qt = q0 + t
                        # qh[p, c, q] = Q[b, qt*128+q, c*128+p]
                        blk = Qh[b, qt * QTILE:(qt + 1) * QTILE, :]
                        qh_a[qslot + t] = blk.reshape(
                            QTILE, DCH, 128).transpose(2, 1, 0)
                    klen = int(min(max(kl_i[b], 1), KT))
                    k0 = 0
                    for j, w in enumerate(ws):
                        if k0 >= nk[b] * 128:
                            break
                        # kv[p, c, k] = K[b, k0+k, c*128+p]  (c < DCH)
                        blk = Kh[b, k0:k0 + w, :]
                        kv_a[chslot + j, :, :DCH, :w] = blk.reshape(
                            w, DCH, 128).transpose(2, 1, 0)
                        # kv[p, DCH+1+2*kb+dh, d] = V[b, k0+kb*128+p, dh*512+d]
                        nb = w // 128
                        blk = Vr[b, k0:k0 + w, :]
                        kv_a[chslot + j, :, DCH + 1:DCH + 1 + 2 * nb, :] = \
                            blk.reshape(nb, 128, 2, KCH).transpose(
                                1, 0, 2, 3).reshape(128, 2 * nb, KCH)
                        kk = np.arange(k0, k0 + w)
                        kv_a[chslot + j, :, DCH, :w] = np.where(
                            kk < klen, 0.0, MASKVAL).astype(np.float16)[None, :]
                        k0 += w
                qslot += NQ
                chslot += len(ws)
        return {"qh": qh_a, "kv": kv_a,
                "ident": np.eye(128, dtype=np.float16)}

    def unpack_core(run_for, o_a, out, done):
        qslot = 0
        for gi, (KB, NQ, F) in enumerate(groups):
            for f in range(F):
                run = run_for(gi, f)
                if run is not None:
                    b, q0, ln = run
                    for t in range(ln):
                        out[b, (q0 + t) * QTILE:(q0 + t + 1) * QTILE, :] = \
                            o_a[qslot + t].astype(np.float32)
                        done[b, q0 + t] = True
                qslot += NQ

    out = np.empty((B, QT, D), dtype=np.float32)
    v_mean = V.mean(axis=1, dtype=np.float64).astype(np.float32)  # [B, D]
    done = np.zeros((B, QT // QTILE), dtype=bool)

    in_maps = [
        pack_core(lambda gi, f, cc=c: assign.get((cc, gi, f)))
        for c in range(NCORES)
    ]
    results = _execute(nc, in_maps)
    for c in range(NCORES):
        unpack_core(lambda gi, f, cc=c: assign.get((cc, gi, f)),
                    results[c]["o"], out, done)

    # rows q >= Q_len: reference yields uniform average over ALL of V
    for b in range(B):
        qlen = int(min(max(ql_i[b], 0), QT))
        out[b, qlen:, :] = v_mean[b]
        assert done[b, :nqt[b]].all()
    return out


# revision 63
# speedup vs baseline: 1.0296x; 1.0296x over previous
"""Masked attention (B=16, QT=KT=2048, D=1024, fp32) on 8 Trainium2 NeuronCores.

Strategy:
 - Work unit = (128 q rows) x (up to 512 k cols) partial attention with online
   (flash-style) softmax accumulation; k-outer / q-inner inside a "fragment".
 - K-length specialization at 128-col granularity: a fragment processing a
   batch with KB = ceil(K_len/128) 128-blocks executes full 512-wide chunks
   plus one final variable-width chunk (128/256/384), cutting both matmul rows
   and K/V DMA to the actual length.
 - A fragment = (NQ q-tiles) x (KB k-blocks) of one batch; every core runs an
   identical static sequence of fragment shapes (SPMD), host packs which
   (batch, q-range) goes where, padding with dummy slots. The slot-shape plan
   comes from a DFS that minimizes max(PE time, DMA time): PE pays KB*NQ
   k128-units per slot (incl. padding), DMA pays KB chunk-loads.
 - Per-chunk DRAM record [128 part, 17, 512] fp16 = 8 K d-chunks + mask row
   + 8 V (kb, dh) columns, packed host-side partition-major so every DMA line
   is 2-17KB contiguous (few, large packets), split into a k+mask DMA (gates
   QK) and a V DMA (gates PV only).
 - Engines: PE does QK / P-transpose / PV matmuls only; DVE adds the mask
   (draining S PSUM), takes the running max, and rescales the O accumulator;
   Scalar does exp (+row sums) and finalization; Pool only dispatches Q DMAs
   (its compute datapath is a slow software DSP - do not offload math there;
   it also cannot touch PSUM).
 - Numerics: Q/K/V/P in fp16 (S accumulated in fp32 PSUM, softmax in fp32),
   output fp16 (quantization ~5e-4 vs |O|max, halves output DMA). End-to-end
   absmax relative error vs the fp32 reference ~1.1e-2 (gate is 2e-2).
   Invalid q rows fixed on host (uniform average of V).
"""

import os
import numpy as np
from contextlib import ExitStack

import concourse.bass as bass
import concourse.tile as tile
from concourse import bacc, mybir
from concourse.bass_utils import run_bass_kernel_spmd

F32 = mybir.dt.float32
FP16 = mybir.dt.float16
AF = mybir.ActivationFunctionType
ALU = mybir.AluOpType

B, QT, KT, D = 16, 2048, 2048, 1024
QTILE = 128
KCH = 512               # max chunk width
NCORES = 8
DCH = D // 128          # 8 contraction chunks of 128
KBLK = KCH // 128       # max k sub-blocks per chunk
MASKVAL = -60000.0      # fp16-exact; exp(S+mask-m) underflows to 0 exactly
NQ_MAX = 8
WARMUP_MM = 26

_PROG_CACHE: dict = {}
LAST_EXEC_NS = [None]

# planner time model (ns): per k128-unit PE, per k128-chunk DMA, fixed terms
PE_UNIT_NS = 920.0
DMA_CHUNK_NS = float(os.environ.get("ATTN_DMA_CHUNK_NS", "1750"))
PE_FIXED_NS = 4500.0
DMA_FIXED_NS = 28000.0


def _widths(kb):
    """Chunk widths (in cols) for kb 128-blocks: full 512s + one partial."""
    ws = [KCH] * (kb // KBLK)
    if kb % KBLK:
        ws.append(128 * (kb % KBLK))
    return ws


def _group_widths(gi, kb):
    """Chunk widths for group gi (uniform; a leading-chunk split was tried
    to cut the first QK's DMA wait but regressed ~3.6us: the extra chunk's
    per-tile softmax instances and fragmented early DMA cost more)."""
    return _widths(kb)


# --------------------------------------------------------------------------
# planning: choose fragment shape classes + assign (batch, q-run) fragments
# --------------------------------------------------------------------------

def _fill_slot(kb, nq, rem, nk, min_frac=0.0):
    """Greedily fill the NCORES positions of a (kb, nq) slot from remaining
    tile demand (batches with nk <= kb, largest covered-work first). A first
    pass only admits batches with nk >= min_frac*kb (avoids stranding deep
    batches while shallow ones steal deep slots); leftover positions are
    filled unrestricted. Returns (placed [(b, ln)...], new_rem)."""
    rem = dict(rem)
    placed = []
    for lo in (min_frac * kb, 0.0) if min_frac else (0.0,):
        while len(placed) < NCORES:
            cands = [b for b in rem if lo <= nk[b] <= kb]
            if not cands:
                break
            b = max(cands, key=lambda b: (min(rem[b], nq) * nk[b], nk[b]))
            ln = min(rem[b], nq)
            placed.append((b, ln))
            rem[b] -= ln
            if rem[b] == 0:
                del rem[b]
    return placed, rem


def _plan(nqt, nk):
    """DFS over slot shapes: each slot (KB, NQ) is executed by all 8 cores
    (8 positions, one single-batch run of <=NQ tiles per position). The
    objective is span = max(PE time, DMA time): PE pays KB*NQ k128-units per
    slot (incl. padding), DMA pays KB chunk-loads per slot.
    Returns (groups [(KB, NQ, 1)...], assign {(core, gi, 0): (b, q0, ln)})."""
    rem0 = {b: nqt[b] for b in range(len(nqt)) if nqt[b] > 0}
    if not rem0:
        return [], {}
    best = [None, float("inf")]
    nodes = [0]

    def objective(pe, dma):
        pe_ns = PE_UNIT_NS * pe + PE_FIXED_NS
        dma_ns = DMA_CHUNK_NS * dma + DMA_FIXED_NS
        return max(pe_ns, dma_ns) + 0.1 * min(pe_ns, dma_ns)

    def dfs(rem, slots, pe, dma):
        nodes[0] += 1
        if nodes[0] > 300000:
            return
        if not rem:
            obj = objective(pe, dma)
            if obj < best[1]:
                best[0], best[1] = list(slots), obj
            return
        lb_pe = sum(rem[b] * nk[b] for b in rem) / NCORES
        lb_dma = sum(nk[b] * -(-rem[b] // NQ_MAX) for b in rem) / NCORES
        if objective(pe + lb_pe, dma + lb_dma) >= best[1]:
            return
        bmax = max(rem, key=lambda b: (nk[b], rem[b]))
        kb = nk[bmax]
        opts, seen = [], set()
        for nq in range(1, NQ_MAX + 1):
            for mf in (0.0, 0.75, 1.0):
                placed, rem2 = _fill_slot(kb, nq, rem, nk, mf)
                if not placed:
                    continue
                key = tuple(sorted(placed))
                if key in seen:
                    continue
                seen.add(key)
                covered = sum(ln * nk[b] for (b, ln) in placed)
                score = covered / (kb * nq + 1.9 * kb)
                opts.append((score, nq, placed, rem2))
        opts.sort(reverse=True, key=lambda o: o[0])
        for (_, nq, placed, rem2) in opts:
            dfs(rem2, slots + [(kb, nq, placed)], pe + kb * nq, dma + kb)

    dfs(rem0, [], 0.0, 0.0)
    assert best[0] is not None, "no feasible plan"

    # the most compute-dense slot (highest NQ) first to prime the pipeline
    # (most PE work per DMA byte while queues ramp), then large-KB first;
    # small fragments at the tail ride on already-prefetched DMA
    slots = sorted(best[0], key=lambda s: (-s[0], -s[1]))
    lead = max(range(len(slots)), key=lambda i: (slots[i][1], slots[i][0]))
    slots = [slots[lead]] + slots[:lead] + slots[lead + 1:]
    groups = [(kb, nq, 1) for (kb, nq, _) in slots]

    # convert per-slot (b, ln) lists into runs with q offsets, then assign
    # positions to cores balancing cumulative real work
    offs = {b: 0 for b in rem0}
    load = [0.0] * NCORES
    assign = {}
    for gi, (kb, nq, placed) in enumerate(slots):
        runs = []
        for (b, ln) in placed:
            runs.append((b, offs[b], ln))
            offs[b] += ln
        runs.sort(key=lambda r: -(r[2] * nk[r[0]]))
        order = sorted(range(NCORES), key=lambda c: load[c])
        for i, (b, q0, ln) in enumerate(runs):
            c = order[i]
            assign[(c, gi, 0)] = (b, q0, ln)
            load[c] += ln * nk[b]
    return groups, assign


# --------------------------------------------------------------------------
# device program (cached by fragment-shape signature)
# --------------------------------------------------------------------------

def _build_program(groups):
    TQ = sum(NQ * F for (_, NQ, F) in groups)
    CH = sum(len(_group_widths(gi, KB)) * F
             for gi, (KB, _, F) in enumerate(groups))

    nc = bacc.Bacc("TRN2", target_bir_lowering=False, debug=False)
    # partition-major contiguous layouts: per partition line 2-17KB.
    # kv record free layout: [0:8]=K d-chunks, [8]=mask row, [9:17]=V pairs
    qh_e = nc.dram_tensor("qh", [TQ, 128, DCH, QTILE], FP16, kind="ExternalInput")
    kv_e = nc.dram_tensor("kv", [CH, 128, 2 * DCH + 1, KCH], FP16, kind="ExternalInput")
    id_e = nc.dram_tensor("ident", [128, 128], FP16, kind="ExternalInput")
    o_e = nc.dram_tensor("o", [TQ, 128, D], FP16, kind="ExternalOutput")

    with tile.TileContext(nc) as tc:
        with ExitStack() as ctx:
            const = ctx.enter_context(tc.tile_pool(name="const", bufs=1))
            deep = 6
            qpool = ctx.enter_context(tc.tile_pool(name="qpool", bufs=3))
            kvpool = ctx.enter_context(tc.tile_pool(name="kvpool", bufs=deep))
            state = ctx.enter_context(tc.tile_pool(name="state", bufs=2))
            work = ctx.enter_context(tc.tile_pool(name="work", bufs=3))
            small = ctx.enter_context(tc.tile_pool(name="small", bufs=6))
            opool = ctx.enter_context(tc.tile_pool(name="opool", bufs=2))
            ps_s = ctx.enter_context(tc.tile_pool(name="ps_s", bufs=2, space="PSUM"))
            ps_t = ctx.enter_context(tc.tile_pool(name="ps_t", bufs=2, space="PSUM"))
            ps_o = ctx.enter_context(tc.tile_pool(name="ps_o", bufs=2, space="PSUM"))

            ident = const.tile([128, 128], FP16)
            nc.sync.dma_start(ident[:], id_e[:])
            # HAM warm-up: dummy matmuls during the initial DMA ramp so the
            # PE clock is at full p-state when real work starts
            for w in range(WARMUP_MM):
                wp = ps_t.tile([128, 128], F32, tag="ptp")
                nc.tensor.matmul(wp[:], ident[:], ident[:], start=True,
                                 stop=True)

            qslot = 0
            chslot = 0
            for gi, (KB, NQ, F) in enumerate(groups):
                ws = _group_widths(gi, KB)
                NKC = len(ws)
                for f in range(F):
                    # fragment state (not needed for single-chunk fragments)
                    if NKC > 1:
                        mbar = state.tile([128, NQ], F32, tag="mbar")
                        dst = state.tile([128, NQ], F32, tag="dst")
                        oacc = state.tile([128, NQ * D], F32, tag="oacc")

                    # load this fragment's q tiles
                    qh = qpool.tile([128, NQ, DCH, QTILE], FP16, tag="qh")
                    for t in range(NQ):
                        nc.gpsimd.dma_start(qh[:, t], qh_e[qslot + t])

                    for j, w in enumerate(ws):
                        nb = w // 128
                        # k+mask and v as separate DMAs: QK/mask-add only
                        # wait on the first, PV on the second
                        kv = kvpool.tile([128, 2 * DCH + 1, KCH], FP16, tag="kv")
                        if w == KCH:
                            nc.sync.dma_start(kv[:, :DCH + 1],
                                              kv_e[chslot + j][:, :DCH + 1])
                            nc.sync.dma_start(kv[:, DCH + 1:],
                                              kv_e[chslot + j][:, DCH + 1:])
                        else:
                            nc.sync.dma_start(kv[:, :DCH + 1, :w],
                                              kv_e[chslot + j][:, :DCH + 1, :w])
                            nc.sync.dma_start(
                                kv[:, DCH + 1:DCH + 1 + 2 * nb],
                                kv_e[chslot + j][:, DCH + 1:DCH + 1 + 2 * nb])

                        for t in range(NQ):
                            # S accumulation in fp32 PSUM
                            sp = ps_s.tile([128, KCH], F32, tag="sp")
                            for c in range(DCH):
                                nc.tensor.matmul(
                                    sp[:, :w], qh[:, t, c], kv[:, c, :w],
                                    start=(c == 0), stop=(c == DCH - 1))

                            # additive length mask (DVE reads PSUM,
                            # releases the S PSUM bank early)
                            s_sb = work.tile([128, KCH], F32, tag="s_sb")
                            nc.vector.tensor_add(s_sb[:, :w], sp[:, :w],
                                                 kv[:, DCH, :w])
                            mbj = small.tile([128, 1], F32, tag="mbj")
                            nc.vector.tensor_reduce(
                                mbj[:], s_sb[:, :w], axis=mybir.AxisListType.X,
                                op=ALU.max, negate=True)

                            st = slice(t, t + 1)
                            if j == 0:
                                if NKC > 1:
                                    nc.vector.tensor_copy(mbar[:, st], mbj[:])
                                mnew = mbj
                            else:
                                mnew = small.tile([128, 1], F32, tag="mnew")
                                nc.vector.tensor_tensor(
                                    mnew[:], mbj[:], mbar[:, st], ALU.min)
                                alpha = small.tile([128, 1], F32, tag="alpha")
                                # alpha = exp(m_old - m_new) = exp(mnew_bar - mold_bar)
                                nc.scalar.activation(
                                    alpha[:], mbar[:, st], AF.Exp,
                                    bias=mnew[:], scale=-1.0)
                                if j < NKC - 1:
                                    nc.vector.tensor_copy(mbar[:, st], mnew[:])

                            # P = exp(S - m), row sums
                            p_sb = work.tile([128, KCH], FP16, tag="p_sb")
                            sj = small.tile([128, 1], F32, tag="sj")
                            nc.scalar.activation(
                                p_sb[:, :w], s_sb[:, :w], AF.Exp, bias=mnew[:],
                                scale=1.0, accum_out=sj[:])

                            if NKC > 1:
                                if j == 0:
                                    nc.vector.tensor_copy(dst[:, st], sj[:])
                                else:
                                    nc.vector.scalar_tensor_tensor(
                                        out=dst[:, st], in0=dst[:, st],
                                        scalar=alpha[:], in1=sj[:],
                                        op0=ALU.mult, op1=ALU.add)

                            # transpose P blocks; per-block PSUM->SBUF copies
                            # so PV's first matmul starts after block 0 only
                            # (a single batched copy measured 2.9us slower)
                            pt = work.tile([128, KBLK, 128], FP16, tag="pt")
                            for kb in range(nb):
                                ptp = ps_t.tile([128, 128], FP16, tag="ptp")
                                nc.tensor.transpose(
                                    ptp[:], p_sb[:, bass.ts(kb, 128)], ident[:])
                                nc.vector.tensor_copy(pt[:, kb], ptp[:])

                            # O_j = P^T-blocks @ V
                            op = ps_o.tile([128, D], F32, tag="op")
                            for dh in range(2):
                                for kb in range(nb):
                                    nc.tensor.matmul(
                                        op[:, bass.ds(dh * 512, 512)],
                                        pt[:, kb],
                                        kv[:, DCH + 1 + 2 * kb + dh],
                                        start=(kb == 0), stop=(kb == nb - 1))

                            ot = slice(t * D, (t + 1) * D)
                            if NKC == 1:
                                # single-chunk fragment: finalize straight
                                # from PSUM (no accumulator round-trip)
                                rec = small.tile([128, 1], F32, tag="rec")
                                nc.vector.reciprocal(rec[:], sj[:])
                                ofin = opool.tile([128, D], FP16, tag="ofin")
                                nc.scalar.activation(
                                    ofin[:], op[:], AF.Copy, bias=0.0,
                                    scale=rec[:])
                                nc.sync.dma_start(o_e[qslot + t], ofin[:])
                                continue
                            if j == 0:
                                nc.scalar.copy(oacc[:, ot], op[:])
                            else:
                                nc.vector.scalar_tensor_tensor(
                                    out=oacc[:, ot], in0=oacc[:, ot],
                                    scalar=alpha[:], in1=op[:],
                                    op0=ALU.mult, op1=ALU.add)
                            if j == NKC - 1:
                                # finalize this q-tile now: overlaps with the
                                # remaining tiles' compute instead of stacking
                                # at the fragment end
                                rec = small.tile([128, 1], F32, tag="rec")
                                nc.vector.reciprocal(rec[:], dst[:, st])
                                ofin = opool.tile([128, D], FP16, tag="ofin")
                                nc.scalar.activation(
                                    ofin[:], oacc[:, ot], AF.Copy, bias=0.0,
                                    scale=rec[:])
                                nc.sync.dma_start(o_e[qslot + t], ofin[:])

                    qslot += NQ
                    chslot += NKC

    nc.compile()
    return nc, TQ, CH


# --------------------------------------------------------------------------
# cached PJRT executor (adapted from concourse.bass2jax.run_bass_via_pjrt)
# --------------------------------------------------------------------------

_EXEC_CACHE: dict = {}


def _get_exec(nc):
    import jax
    from concourse import bass2jax, mybir as _mb
    from jax.experimental.shard_map import shard_map
    from jax.sharding import Mesh, PartitionSpec

    key = id(nc)
    if key in _EXEC_CACHE:
        return _EXEC_CACHE[key]
    bass2jax.install_neuronx_cc_hook()
    assert not nc.dbg_addr or not nc.dbg_callbacks

    partition_name = nc.partition_id_tensor.name if nc.partition_id_tensor else None
    in_names, out_names, out_avals = [], [], []
    for alloc in nc.m.functions[0].allocations:
        if not isinstance(alloc, _mb.MemoryLocationSet):
            continue
        name = alloc.memorylocations[0].name
        if alloc.kind == "ExternalInput":
            if name != partition_name:
                in_names.append(name)
        elif alloc.kind == "ExternalOutput":
            shape = tuple(alloc.tensor_shape)
            dtype = _mb.dt.np(alloc.dtype)
            out_names.append(name)
            out_avals.append(jax.core.ShapedArray(shape, dtype))
    n_params = len(in_names)
    n_outs = len(out_avals)
    all_in_names = list(in_names) + list(out_names)
    if partition_name is not None:
        all_in_names.append(partition_name)
    donate = tuple(range(n_params, n_params + n_outs))

    def _body(*args):
        operands = list(args)
        if partition_name is not None:
            operands.append(bass2jax.partition_id_tensor())
        return tuple(bass2jax._bass_exec_p.bind(
            *operands,
            out_avals=tuple(out_avals),
            in_names=tuple(all_in_names),
            out_names=tuple(out_names),
            lowering_input_output_aliases=(),
            sim_require_finite=True,
            sim_require_nnan=True,
            nc=nc,
        ))

    devices = jax.devices()[:NCORES]
    mesh = Mesh(np.asarray(devices), ("core",))
    in_specs = (PartitionSpec("core"),) * (n_params + n_outs)
    out_specs = (PartitionSpec("core"),) * n_outs
    sharded = jax.jit(
        shard_map(_body, mesh=mesh, in_specs=in_specs, out_specs=out_specs,
                  check_rep=False),
        donate_argnums=donate, keep_unused=True)
    info = dict(sharded=sharded, in_names=in_names, out_names=out_names,
                out_avals=out_avals, mesh=mesh, n_params=n_params)
    _EXEC_CACHE[key] = info
    return info


def _concat_inputs(info, in_maps):
    return [np.concatenate([np.asarray(m[name]) for m in in_maps], axis=0)
            for name in info["in_names"]]


def _zero_outs(info):
    return [np.zeros((NCORES * a.shape[0], *a.shape[1:]), a.dtype)
            for a in info["out_avals"]]


def _execute(nc, in_maps):
    try:
        info = _get_exec(nc)
        concat_in = _concat_inputs(info, in_maps)
        out_arrs = info["sharded"](*concat_in, *_zero_outs(info))
        results = [
            {name: np.asarray(out_arrs[i]).reshape(
                NCORES, *info["out_avals"][i].shape)[c]
             for i, name in enumerate(info["out_names"])}
            for c in range(NCORES)
        ]
        if int(os.environ.get("ATTN_TIME", "0")):
            LAST_EXEC_NS[0] = _time_exec(
                nc, concat_in, int(os.environ.get("ATTN_TIME_ITERS", "3")))
        return results
    except Exception:
        # robust fallback: the canonical entry point (same underlying path,
        # uncached) — also covers non-axon native environments
        res = run_bass_kernel_spmd(nc, in_maps, core_ids=list(range(NCORES)))
        return res.results


def _time_exec(nc, concat_in, iters=3):
    """Wall-clock the sharded execution with device-resident inputs."""
    import time
    import jax
    from jax.sharding import NamedSharding, PartitionSpec
    info = _get_exec(nc)
    sh = NamedSharding(info["mesh"], PartitionSpec("core"))
    dev_in = [jax.device_put(x, sh) for x in concat_in]
    for x in dev_in:
        x.block_until_ready()
    times = []
    for _ in range(iters):
        zeros = [jax.device_put(z, sh) for z in _zero_outs(info)]
        for z in zeros:
            z.block_until_ready()
        t0 = time.perf_counter()
        outs = info["sharded"](*dev_in, *zeros)
        for o in outs:
            o.block_until_ready()
        times.append(time.perf_counter() - t0)
    best = min(times)
    print(f"exec wall times: {[f'{t*1e3:.2f}ms' for t in times]}")
    return int(best * 1e9)


# --------------------------------------------------------------------------
# host entry
# --------------------------------------------------------------------------

def kernel(Q, K, V, Q_lengths, K_lengths):
    Q = np.ascontiguousarray(np.asarray(Q, dtype=np.float32))
    K = np.ascontiguousarray(np.asarray(K, dtype=np.float32))
    V = np.ascontiguousarray(np.asarray(V, dtype=np.float32))
    ql_i = np.asarray(Q_lengths).astype(np.int64)
    kl_i = np.asarray(K_lengths).astype(np.int64)

    nqt = [int(-(-min(max(q, 0), QT) // QTILE)) for q in ql_i]
    nk = [int(-(-min(max(k, 1), KT) // 128)) for k in kl_i]

    groups, assign = _plan(nqt, nk)
    sig = tuple(groups)
    if sig not in _PROG_CACHE:
        _PROG_CACHE[sig] = _build_program(groups)
    nc, TQ, CH = _PROG_CACHE[sig]

    Qh = Q.astype(np.float16)
    Kh = K.astype(np.float16)
    Vr = V.astype(np.float16)

    def pack_core(run_for):
        qh_a = np.zeros((TQ, 128, DCH, QTILE), dtype=np.float16)
        kv_a = np.zeros((CH, 128, 2 * DCH + 1, KCH), dtype=np.float16)
        kv_a[:, :, DCH, :] = MASKVAL
        qslot = chslot = 0
        for gi, (KB, NQ, F) in enumerate(groups):
            ws = _group_widths(gi, KB)
            for f in range(F):
                run = run_for(gi, f)
                if run is not None:
                    b, q0, ln = run
                    for t in range(ln):
                        qt = q0 + t
                        # qh[p, c, q] = Q[b, qt*128+q, c*128+p]
                        blk = Qh[b, qt * QTILE:(qt + 1) * QTILE, :]
                        qh_a[qslot + t] = blk.reshape(
                            QTILE, DCH, 128).transpose(2, 1, 0)
                    klen = int(min(max(kl_i[b], 1), KT))
                    k0 = 0
                    for j, w in enumerate(ws):
                        if k0 >= nk[b] * 128:
                            break
                        # kv[p, c, k] = K[b, k0+k, c*128+p]  (c < DCH)
                        blk = Kh[b, k0:k0 + w, :]
                        kv_a[chslot + j, :, :DCH, :w] = blk.reshape(
                            w, DCH, 128).transpose(2, 1, 0)
                        # kv[p, DCH+1+2*kb+dh, d] = V[b, k0+kb*128+p, dh*512+d]
                        nb = w // 128
                        blk = Vr[b, k0:k0 + w, :]
                        kv_a[chslot + j, :, DCH + 1:DCH + 1 + 2 * nb, :] = \
                            blk.reshape(nb, 128, 2, KCH).transpose(
                                1, 0, 2, 3).reshape(128, 2 * nb, KCH)
                        kk = np.arange(k0, k0 + w)
                        kv_a[chslot + j, :, DCH, :w] = np.where(
                            kk < klen, 0.0, MASKVAL).astype(np.float16)[None, :]
                        k0 += w
                qslot += NQ
                chslot += len(ws)
        return {"qh": qh_a, "kv": kv_a,
                "ident": np.eye(128, dtype=np.float16)}

    def unpack_core(run_for, o_a, out, done):
        qslot = 0
        for gi, (KB, NQ, F) in enumerate(groups):
            for f in range(F):
                run = run_for(gi, f)
                if run is not None:
                    b, q0, ln = run
                    for t in range(ln):
                        out[b, (q0 + t) * QTILE:(q0 + t + 1) * QTILE, :] = \
                            o_a[qslot + t].astype(np.float32)
                        done[b, q0 + t] = True
                qslot += NQ

    out = np.empty((B, QT, D), dtype=np.float32)
    v_mean = V.mean(axis=1, dtype=np.float64).astype(np.float32)  # [B, D]
    done = np.zeros((B, QT // QTILE), dtype=bool)

    in_maps = [
        pack_core(lambda gi, f, cc=c: assign.get((cc, gi, f)))
        for c in range(NCORES)
    ]
    results = _execute(nc, in_maps)
    for c in range(NCORES):
        unpack_core(lambda gi, f, cc=c: assign.get((cc, gi, f)),
                    results[c]["o"], out, done)

    # rows q >= Q_len: reference yields uniform average over ALL of V
    for b in range(B):
        qlen = int(min(max(ql_i[b], 0), QT))
        out[b, qlen:, :] = v_mean[b]
        assert done[b, :nqt[b]].all()
    return out
